# revision 3
# baseline (speedup 1.0000x reference)
"""TRN2 Bass/Tile kernel for nn_DotProductAttention (softmax over the QUERY axis).

reference:
    scores  = einsum('bqd,bkd->bqk', q, k) / sqrt(64)
    weights = softmax(scores, axis=1)          # over q, NOT k!
    out     = einsum('bqk,bkd->bqd', weights, v)

Works with the transposed score matrix T = K @ Q^T ([k, q]): the softmax
reduction axis (q) is the free axis, and the normalizer Z[k] lives on the
contraction axis of the AV matmul so it folds into V (Vs = V / Z).

Sharding: B=16 batches, data-parallel over 8 cores => 2 batches per core,
packed into the two 64-partition halves of [128, *] tiles.

Per-core structure:
  phase A: f32 quarter loads (q on sync queue, k on scalar queue), bf16
    casts on otherwise-idle DVE/GPS, bounce to DRAM [s, 128] and xbar
    DMA transpose into QT/KT [128 (b d), 2048 s]. V loads f32 via SWDGE.
  phase B: per (k-chunk, batch) tile, 2x N=512 matmuls per q-half into a
    3-slot rotation of [128, 1024] PSUM buffers; ONE full-tile exp per
    tile via a strided [128, 2, 1024] AP over its two (possibly
    non-adjacent) slots. Exp engines are split: ~18 ACT tiles (activation
    table; accum_out = Z for free) and ~14 DVE tiles (Schraudolph
    fast-exp: affine in f32, write the int16 bit pattern of bf16, ~2%
    rms) whose Z is rebuilt by a 2-level GPSIMD half-add tree plus a DVE
    reduce. Per 2-chunk group: batched reciprocal, Vs = V * (1/Z)
    (ACT/DVE split), then the q-half-0 AV accumulation runs in the 2
    spare PSUM banks, overlapped with B1.
  tail: AV for q-half 1 (into a freed PSUM slot), drains, 16 PE
    transposes of O^T, output DMAs.
"""

import math
from contextlib import ExitStack

import numpy as np

import concourse.bass as bass  # noqa: F401
import concourse.mybir as mybir
import concourse.tile as tile
from bass_rust import add_dep_helper
from concourse import bacc, bass_utils
from concourse.masks import make_identity

FP32 = mybir.dt.float32
BF16 = mybir.dt.bfloat16
I16 = mybir.dt.int16

N_CORES = 8
B_FULL = 16
BPC = B_FULL // N_CORES  # batches per core = 2
S = 2048
D = 64
NCH = S // 128  # 16 key chunks of 128
NT = NCH * BPC  # 32 (chunk, batch) tiles
SCALE = 1.0 / math.sqrt(D)

# Schraudolph fast-exp constants for a bf16 bit pattern:
#   bf16_bits(exp(x)) ~= trunc(x * 128/ln2 + (127*128 - 6 + 0.5))
A16 = 128.0 / math.log(2.0)
BIAS16 = 127.0 * 128.0 - 6.0 + 0.5

# DVE-assigned tiles (14 of 32, evenly spread)
DVE_TILES = frozenset({1, 3, 6, 8, 10, 13, 15, 17, 19, 22, 24, 26, 29, 31})


def emit_kernel(ctx: ExitStack, tc, q, k, v, o, qbf_dram, kbf_dram):
    nc = tc.nc

    const_pool = ctx.enter_context(tc.tile_pool(name="const", bufs=1))
    big = ctx.enter_context(tc.tile_pool(name="big", bufs=1))
    # PSUM, manually laid out: PS = 3 rotating [128,1024] score slots
    # (6 banks); pot0 = AV accumulator for q-half 0 (2 banks).
    psp = ctx.enter_context(tc.tile_pool(name="psp", bufs=1, space="PSUM"))
    ppp = ctx.enter_context(tc.tile_pool(name="ppp", bufs=1, space="PSUM"))

    ident = const_pool.tile([128, 128], FP32, name="ident")
    make_identity(nc, ident)
    zw = const_pool.tile([128, 128], BF16, name="zw")
    nc.vector.memset(zw[:], 0.0)

    PS = psp.tile([128, 3072], FP32, name="PS")
    PSv = PS[:].rearrange("p (s c) -> p s c", c=1024)
    pot0 = ppp.tile([128, 1024], FP32, name="pot0")

    # (b,d)-packed transposed operands: partitions 0:64 = batch0 d, 64:128 = b1.
    QT = big.tile([128, S], BF16, name="QT")
    KT = big.tile([128, S], BF16, name="KT")
    # f32 staging ((m b d) columns, s on partitions) and bf16 casts
    qstage = big.tile([128, S], FP32, name="qstage")
    kstage = big.tile([128, S], FP32, name="kstage")
    qbf = big.tile([128, S], BF16, name="qbf")
    kbf = big.tile([128, S], BF16, name="kbf")
    # V chunks [128 k, (t d)] f32 and Vs = V / Z (bf16); t = i*BPC + b
    V = big.tile([128, NT * D], FP32, name="V")
    Vs = big.tile([128, NT * D], BF16, name="Vs")
    # E[t*S :+ S] = exp(scores*SCALE): [128 k, 2048 q] bf16, fully resident.
    # NOTE: within tile t the two q-halves are stored in PSUM-slot order;
    # eoff[t][h] maps q-half h to its byte offset inside the tile.
    E = big.tile([128, NT * S], BF16, name="E")
    Ei16 = E[:].bitcast(I16)
    # per-tile Z and 1/Z
    zs = big.tile([128, NT], FP32, name="zs")
    rz = big.tile([128, NT], FP32, name="rz")
    # gpsimd half-add scratch (2-level tree, rotating pair)
    Tg = big.tile([128, 2 * 1024], FP32, name="Tg")
    Tg2 = big.tile([128, 2 * 512], FP32, name="Tg2")
    # O^T staging ((b,d) packed on partitions, q on free), f32
    OT = big.tile([128, S], FP32, name="OT")
    # O in natural layout: column chunk m holds [q-tile m, (b d)]
    O_all = big.tile([128, S], FP32, name="O_all")

    # ---------------- phase A: loads, casts, bounce, xbar ----------------
    # f32 quarter loads: q on the sync HWDGE queue, k on the scalar queue.
    QRT = NCH // 4
    for Q in range(4):
        ssl = slice(Q * QRT * 128, (Q + 1) * QRT * 128)
        for src, stg, eng in ((q, qstage, nc.sync), (k, kstage, nc.scalar)):
            for b in range(BPC):
                eng.dma_start(
                    stg[:, ssl].rearrange(
                        "p (m b d) -> p m b d", m=QRT, b=BPC, d=D
                    )[:, :, b, :],
                    src[b, ssl, :].rearrange("(m p) d -> p m d", p=128),
                )
    # casts: q quarters on DVE, k quarters on GPS (both idle in phase A)
    for Q in range(4):
        csl = slice(Q * 512, (Q + 1) * 512)
        nc.vector.tensor_copy(qbf[:, csl], qstage[:, csl])
        nc.gpsimd.tensor_copy(kbf[:, csl], kstage[:, csl])
    # bounce to DRAM [s, 128] per half, then xbar transpose back
    for hf in range(2):
        rsl = slice(hf * 1024, (hf + 1) * 1024)
        msl = slice(hf * 8, (hf + 1) * 8)
        nc.sync.dma_start(
            qbf_dram[rsl, :].rearrange("(m p) c -> p m c", p=128),
            qbf[:].rearrange("p (m c) -> p m c", m=NCH)[:, msl, :],
        )
        nc.sync.dma_start_transpose(QT[:, rsl], qbf_dram[rsl, :])
        nc.scalar.dma_start(
            kbf_dram[rsl, :].rearrange("(m p) c -> p m c", p=128),
            kbf[:].rearrange("p (m c) -> p m c", m=NCH)[:, msl, :],
        )
    nc.scalar.dma_start_transpose(KT[:, 0:256], kbf_dram[0:256, :])
    nc.scalar.dma_start_transpose(KT[:, 256:S], kbf_dram[256:S, :])
    # V load (f32), (i b d) column layout, on SWDGE (its own queue)
    for b in range(BPC):
        nc.gpsimd.dma_start(
            V[:].rearrange("p (i b d) -> p i b d", i=NCH, b=BPC)[:, :, b, :],
            v[b].rearrange("(i p) d -> p i d", p=128),
        )

    # pot0: open every (b, j) region with a zeroing matmul so the
    # partition-sliced AV matmuls can accumulate with start=False.
    zmm0 = []
    for j in range(2):
        zmm0.append(
            nc.tensor.matmul(
                pot0[:, j * 512 : (j + 1) * 512],
                lhsT=zw[:],
                rhs=QT[:, 0:512],
                start=True,
                stop=False,
                skip_group_check=True,
            )
        )

    # ---------------- phase B1: scores -> exp (+Z), AV half 0 --------------
    eoff = {}  # (t, h) -> column offset of q-half h inside E tile t

    def emit_av(pot, zmm, t, h, stop_last):
        b = t % BPC
        base = t * S + eoff[(t, h)]
        for j in range(2):
            mm = nc.tensor.matmul(
                pot[b * 64 : (b + 1) * 64, j * 512 : (j + 1) * 512],
                lhsT=Vs[:, t * D : (t + 1) * D],
                rhs=E[:, base + j * 512 : base + (j + 1) * 512],
                start=False,
                stop=stop_last,
                skip_group_check=True,
            )
            if zmm is not None:
                add_dep_helper(
                    mm.ins,
                    zmm[j].ins,
                    sync=False,
                    reason="AV after bank-opening zero matmul",
                )

    for i in range(NCH):
        for b in range(BPC):
            t = i * BPC + b
            sa, sb = (2 * t) % 3, (2 * t + 1) % 3
            mn, mx = min(sa, sb), max(sa, sb)
            eoff[(t, 0)] = 0 if sa == mn else 1024
            eoff[(t, 1)] = 1024 - eoff[(t, 0)]
            for h in range(2):
                slot = sa if h == 0 else sb
                for j in range(2):
                    nc.tensor.matmul(
                        PS[:, slot * 1024 + j * 512 : slot * 1024 + (j + 1) * 512],
                        lhsT=KT[b * 64 : (b + 1) * 64, i * 128 : (i + 1) * 128],
                        rhs=QT[
                            b * 64 : (b + 1) * 64,
                            h * 1024 + j * 512 : h * 1024 + (j + 1) * 512,
                        ],
                        start=True,
                        stop=True,
                    )
            # one full-tile exp over the (strided) slot pair
            pin = PSv[:, mn : mx + 1 : max(mx - mn, 1), :]
            eb = t * S
            eout = E[:, eb : eb + S].rearrange("p (s c) -> p s c", c=1024)
            if t not in DVE_TILES:
                nc.scalar.activation(
                    eout,
                    pin,
                    mybir.ActivationFunctionType.Exp,
                    scale=SCALE,
                    accum_out=zs[:, t : t + 1],
                )
            else:
                nc.vector.tensor_scalar(
                    Ei16[:, eb : eb + S].rearrange("p (s c) -> p s c", c=1024),
                    pin,
                    SCALE * A16,
                    BIAS16,
                    mybir.AluOpType.mult,
                    op1=mybir.AluOpType.add,
                )
                g = t % 2
                nc.gpsimd.tensor_tensor(
                    Tg[:, g * 1024 : (g + 1) * 1024],
                    E[:, eb : eb + 1024],
                    E[:, eb + 1024 : eb + 2048],
                    mybir.AluOpType.add,
                )
                nc.gpsimd.tensor_tensor(
                    Tg2[:, g * 512 : (g + 1) * 512],
                    Tg[:, g * 1024 : g * 1024 + 512],
                    Tg[:, g * 1024 + 512 : (g + 1) * 1024],
                    mybir.AluOpType.add,
                )
                nc.vector.tensor_reduce(
                    zs[:, t : t + 1],
                    Tg2[:, g * 512 : (g + 1) * 512],
                    mybir.AxisListType.X,
                    mybir.AluOpType.add,
                )
        # after odd chunks: finish the 4-tile group (2 chunks)
        if i % 2 == 1:
            g4 = (i - 1) * BPC
            nc.vector.reciprocal(rz[:, g4 : g4 + 4], zs[:, g4 : g4 + 4])
            for t in range(g4, g4 + 4):
                # V scaling: DVE tiles' Vs on ACT, ACT tiles' Vs on DVE
                if t in DVE_TILES:
                    nc.scalar.mul(
                        Vs[:, t * D : (t + 1) * D],
                        V[:, t * D : (t + 1) * D],
                        rz[:, t : t + 1],
                    )
                else:
                    nc.vector.tensor_scalar_mul(
                        Vs[:, t * D : (t + 1) * D],
                        V[:, t * D : (t + 1) * D],
                        rz[:, t : t + 1],
                    )
            for t in range(g4, g4 + 4):
                emit_av(pot0, zmm0, t, 0, stop_last=(t == NT - 1))

    # ---------------- tail: AV half 1, drains, transposes, stores ----------
    # pot1 reuses PS slot 0 (cols 0:1024); transpose scratch uses slot 2.
    pot1 = PS[:, 0:1024]
    zmm1 = []
    for j in range(2):
        zmm1.append(
            nc.tensor.matmul(
                pot1[:, j * 512 : (j + 1) * 512],
                lhsT=zw[:],
                rhs=QT[:, 0:512],
                start=True,
                stop=False,
                skip_group_check=True,
            )
        )
    # j=0 AVs for all tiles, then j=1 (so the j=0 drain can start early)
    for j in range(2):
        for t in range(NT):
            b = t % BPC
            base = t * S + eoff[(t, 1)]
            mm = nc.tensor.matmul(
                pot1[b * 64 : (b + 1) * 64, j * 512 : (j + 1) * 512],
                lhsT=Vs[:, t * D : (t + 1) * D],
                rhs=E[:, base + j * 512 : base + (j + 1) * 512],
                start=False,
                stop=(t == NT - 1),
                skip_group_check=True,
            )
            add_dep_helper(
                mm.ins, zmm1[j].ins, sync=False, reason="AV after zero matmul"
            )

    # drains: pot0 (free after its last AV), then pot1 halves
    for c in range(2):
        nc.scalar.copy(OT[:, c * 512 : (c + 1) * 512], pot0[:, c * 512 : (c + 1) * 512])
    for c in range(2):
        nc.scalar.copy(
            OT[:, 1024 + c * 512 : 1024 + (c + 1) * 512],
            pot1[:, c * 512 : (c + 1) * 512],
        )

    o_view = O_all[:].rearrange("p (m b d) -> p m b d", m=NCH, b=BPC, d=D)
    for grp in range(4):
        for m in range(4 * grp, 4 * grp + 4):
            ptc = PS[:, 2048 + (m % 4) * 128 : 2048 + (m % 4) * 128 + 128]
            nc.tensor.transpose(ptc, OT[:, m * 128 : (m + 1) * 128], ident[:])
            if m % 2 == 0:
                nc.vector.tensor_copy(O_all[:, m * 128 : (m + 1) * 128], ptc)
            else:
                nc.scalar.copy(O_all[:, m * 128 : (m + 1) * 128], ptc)
        for b in range(BPC):
            nc.sync.dma_start(
                o[b, 4 * grp * 128 : (4 * grp + 4) * 128, :].rearrange(
                    "(m p) d -> p m d", p=128
                ),
                o_view[:, 4 * grp : 4 * grp + 4, b, :],
            )


_CACHE: dict = {}


def build_program():
    if "nc" in _CACHE:
        return _CACHE["nc"]
    nc = bacc.Bacc("TRN2", target_bir_lowering=False, debug=False)
    q = nc.dram_tensor("q", [BPC, S, D], FP32, kind="ExternalInput").ap()
    k = nc.dram_tensor("k", [BPC, S, D], FP32, kind="ExternalInput").ap()
    v = nc.dram_tensor("v", [BPC, S, D], FP32, kind="ExternalInput").ap()
    o = nc.dram_tensor("o", [BPC, S, D], FP32, kind="ExternalOutput").ap()
    qbf_dram = nc.dram_tensor("qbf_dram", [S, 128], BF16, kind="Internal").ap()
    kbf_dram = nc.dram_tensor("kbf_dram", [S, 128], BF16, kind="Internal").ap()
    with tile.TileContext(nc) as tc:
        with ExitStack() as ctx:
            emit_kernel(ctx, tc, q, k, v, o, qbf_dram, kbf_dram)
    nc.compile()
    _CACHE["nc"] = nc
    return nc


def make_in_maps(q, k, v):
    q = np.ascontiguousarray(q, dtype=np.float32)
    k = np.ascontiguousarray(k, dtype=np.float32)
    v = np.ascontiguousarray(v, dtype=np.float32)
    assert q.shape == (B_FULL, S, D), q.shape
    return [
        {
            "q": np.ascontiguousarray(q[c * BPC : (c + 1) * BPC]),
            "k": np.ascontiguousarray(k[c * BPC : (c + 1) * BPC]),
            "v": np.ascontiguousarray(v[c * BPC : (c + 1) * BPC]),
        }
        for c in range(N_CORES)
    ]


def kernel(q, k, v, _trace=False):
    nc = build_program()
    in_maps = make_in_maps(q, k, v)
    res = bass_utils.run_bass_kernel_spmd(
        nc, in_maps, core_ids=list(range(N_CORES)), trace=_trace
    )
    out = np.concatenate([r["o"] for r in res.results], axis=0)
    if _trace:
        return out, res
    return out


# revision 6
# speedup vs baseline: 1.0056x; 1.0056x over previous
"""TRN2 Bass/Tile kernel for nn_DotProductAttention (softmax over the QUERY axis).

reference:
    scores  = einsum('bqd,bkd->bqk', q, k) / sqrt(64)
    weights = softmax(scores, axis=1)          # over q, NOT k!
    out     = einsum('bqk,bkd->bqd', weights, v)

Works with the transposed score matrix T = K @ Q^T ([k, q]): the softmax
reduction axis (q) is the free axis, and the normalizer Z[k] lives on the
contraction axis of the AV matmul so it folds into V (Vs = V / Z).

Sharding: B=16 batches, data-parallel over 8 cores => 2 batches per core,
packed into the two 64-partition halves of [128, *] tiles.

Per-core structure:
  phase A: f32 quarter loads (q on sync queue, k on scalar queue), bf16
    casts on otherwise-idle DVE/GPS, bounce to DRAM [s, 128] and xbar
    DMA transpose into QT/KT [128 (b d), 2048 s]. V loads f32 via SWDGE.
  phase B: per (k-chunk, batch) tile, 2x N=512 matmuls per q-half into a
    3-slot rotation of [128, 1024] PSUM buffers; ONE full-tile exp per
    tile via a strided [128, 2, 1024] AP over its two (possibly
    non-adjacent) slots. Exp engines are split: ~18 ACT tiles (activation
    table; accum_out = Z for free) and ~14 DVE tiles (Schraudolph
    fast-exp: affine in f32, write the int16 bit pattern of bf16, ~2%
    rms) whose Z is rebuilt by a 2-level GPSIMD half-add tree plus a DVE
    reduce. Per 2-chunk group: batched reciprocal, Vs = V * (1/Z)
    (ACT/DVE split), then the q-half-0 AV accumulation runs in the 2
    spare PSUM banks, overlapped with B1.
  tail: AV for q-half 1 (into a freed PSUM slot), drains, 16 PE
    transposes of O^T, output DMAs.
"""

import math
from contextlib import ExitStack

import numpy as np

import concourse.bass as bass  # noqa: F401
import concourse.mybir as mybir
import concourse.tile as tile
from bass_rust import add_dep_helper
from concourse import bacc, bass_utils
from concourse.masks import make_identity

FP32 = mybir.dt.float32
BF16 = mybir.dt.bfloat16
I16 = mybir.dt.int16

N_CORES = 8
B_FULL = 16
BPC = B_FULL // N_CORES  # batches per core = 2
S = 2048
D = 64
NCH = S // 128  # 16 key chunks of 128
NT = NCH * BPC  # 32 (chunk, batch) tiles
SCALE = 1.0 / math.sqrt(D)

# Schraudolph fast-exp constants for a bf16 bit pattern:
#   bf16_bits(exp(x)) ~= trunc(x * 128/ln2 + (127*128 - 6 + 0.5))
A16 = 128.0 / math.log(2.0)
BIAS16 = 127.0 * 128.0 - 6.0 + 0.5

# DVE-assigned tiles (14 of 32, evenly spread)
DVE_TILES = frozenset({1, 3, 6, 8, 10, 13, 15, 17, 19, 22, 24, 26, 29, 31})


def emit_kernel(ctx: ExitStack, tc, q, k, v, o, qbf_dram, kbf_dram):
    nc = tc.nc

    const_pool = ctx.enter_context(tc.tile_pool(name="const", bufs=1))
    big = ctx.enter_context(tc.tile_pool(name="big", bufs=1))
    # PSUM, manually laid out: PS = 3 rotating [128,1024] score slots
    # (6 banks); pot0 = AV accumulator for q-half 0 (2 banks).
    psp = ctx.enter_context(tc.tile_pool(name="psp", bufs=1, space="PSUM"))
    ppp = ctx.enter_context(tc.tile_pool(name="ppp", bufs=1, space="PSUM"))

    ident = const_pool.tile([128, 128], FP32, name="ident")
    make_identity(nc, ident)
    zw = const_pool.tile([128, 128], BF16, name="zw")
    nc.vector.memset(zw[:], 0.0)

    PS = psp.tile([128, 3072], FP32, name="PS")
    PSv = PS[:].rearrange("p (s c) -> p s c", c=1024)
    pot0 = ppp.tile([128, 1024], FP32, name="pot0")

    # (b,d)-packed transposed operands: partitions 0:64 = batch0 d, 64:128 = b1.
    QT = big.tile([128, S], BF16, name="QT")
    KT = big.tile([128, S], BF16, name="KT")
    # f32 staging ((m b d) columns, s on partitions) and bf16 casts
    qstage = big.tile([128, S], FP32, name="qstage")
    kstage = big.tile([128, S], FP32, name="kstage")
    qbf = big.tile([128, S], BF16, name="qbf")
    kbf = big.tile([128, S], BF16, name="kbf")
    # V chunks [128 k, (t d)] f32 and Vs = V / Z (bf16); t = i*BPC + b
    V = big.tile([128, NT * D], FP32, name="V")
    Vs = big.tile([128, NT * D], BF16, name="Vs")
    # E[t*S :+ S] = exp(scores*SCALE): [128 k, 2048 q] bf16, fully resident.
    # NOTE: within tile t the two q-halves are stored in PSUM-slot order;
    # eoff[t][h] maps q-half h to its byte offset inside the tile.
    E = big.tile([128, NT * S], BF16, name="E")
    Ei16 = E[:].bitcast(I16)
    # per-tile Z and 1/Z (zc2: second-half accum scratch for wrap tiles)
    zs = big.tile([128, NT], FP32, name="zs")
    rz = big.tile([128, NT], FP32, name="rz")
    zc2 = big.tile([128, 2], FP32, name="zc2")
    # gpsimd half-add scratch (2-level tree, rotating pair)
    Tg = big.tile([128, 2 * 1024], FP32, name="Tg")
    Tg2 = big.tile([128, 2 * 512], FP32, name="Tg2")
    # O^T staging ((b,d) packed on partitions, q on free), f32
    OT = big.tile([128, S], FP32, name="OT")
    # O in natural layout: column chunk m holds [q-tile m, (b d)]
    O_all = big.tile([128, S], FP32, name="O_all")

    # ---------------- phase A: loads, casts, bounce, xbar ----------------
    # f32 quarter loads: q on the sync HWDGE queue, k on the scalar queue.
    QRT = NCH // 4
    for Q in range(4):
        ssl = slice(Q * QRT * 128, (Q + 1) * QRT * 128)
        for src, stg, eng in ((q, qstage, nc.sync), (k, kstage, nc.scalar)):
            for b in range(BPC):
                eng.dma_start(
                    stg[:, ssl].rearrange(
                        "p (m b d) -> p m b d", m=QRT, b=BPC, d=D
                    )[:, :, b, :],
                    src[b, ssl, :].rearrange("(m p) d -> p m d", p=128),
                )
    # casts: all on DVE (idle in phase A; GPS's copy ucode is ~6x slower)
    for Q in range(4):
        csl = slice(Q * 512, (Q + 1) * 512)
        nc.vector.tensor_copy(qbf[:, csl], qstage[:, csl])
        nc.vector.tensor_copy(kbf[:, csl], kstage[:, csl])
    # bounce to DRAM [s, 128] per half, then xbar transpose back
    for hf in range(2):
        rsl = slice(hf * 1024, (hf + 1) * 1024)
        msl = slice(hf * 8, (hf + 1) * 8)
        nc.sync.dma_start(
            qbf_dram[rsl, :].rearrange("(m p) c -> p m c", p=128),
            qbf[:].rearrange("p (m c) -> p m c", m=NCH)[:, msl, :],
        )
        nc.sync.dma_start_transpose(QT[:, rsl], qbf_dram[rsl, :])
        nc.scalar.dma_start(
            kbf_dram[rsl, :].rearrange("(m p) c -> p m c", p=128),
            kbf[:].rearrange("p (m c) -> p m c", m=NCH)[:, msl, :],
        )
    nc.scalar.dma_start_transpose(KT[:, 0:256], kbf_dram[0:256, :])
    nc.scalar.dma_start_transpose(KT[:, 256:S], kbf_dram[256:S, :])
    # V load (f32), (i b d) column layout, on SWDGE (its own queue)
    for b in range(BPC):
        nc.gpsimd.dma_start(
            V[:].rearrange("p (i b d) -> p i b d", i=NCH, b=BPC)[:, :, b, :],
            v[b].rearrange("(i p) d -> p i d", p=128),
        )

    # pot0: open every (b, j) region with a zeroing matmul so the
    # partition-sliced AV matmuls can accumulate with start=False.
    zmm0 = []
    for j in range(2):
        zmm0.append(
            nc.tensor.matmul(
                pot0[:, j * 512 : (j + 1) * 512],
                lhsT=zw[:],
                rhs=QT[:, 0:512],
                start=True,
                stop=False,
                skip_group_check=True,
            )
        )

    # ---------------- phase B1: scores -> exp (+Z), AV half 0 --------------
    eoff = {}  # (t, h) -> column offset of q-half h inside E tile t

    def emit_av(pot, zmm, t, h, stop_last):
        b = t % BPC
        base = t * S + eoff[(t, h)]
        for j in range(2):
            mm = nc.tensor.matmul(
                pot[b * 64 : (b + 1) * 64, j * 512 : (j + 1) * 512],
                lhsT=Vs[:, t * D : (t + 1) * D],
                rhs=E[:, base + j * 512 : base + (j + 1) * 512],
                start=False,
                stop=stop_last,
                skip_group_check=True,
            )
            if zmm is not None:
                add_dep_helper(
                    mm.ins,
                    zmm[j].ins,
                    sync=False,
                    reason="AV after bank-opening zero matmul",
                )

    for i in range(NCH):
        for b in range(BPC):
            t = i * BPC + b
            sa, sb = (2 * t) % 3, (2 * t + 1) % 3
            mn, mx = min(sa, sb), max(sa, sb)
            eoff[(t, 0)] = 0 if sa == mn else 1024
            eoff[(t, 1)] = 1024 - eoff[(t, 0)]
            for h in range(2):
                slot = sa if h == 0 else sb
                for j in range(2):
                    nc.tensor.matmul(
                        PS[:, slot * 1024 + j * 512 : slot * 1024 + (j + 1) * 512],
                        lhsT=KT[b * 64 : (b + 1) * 64, i * 128 : (i + 1) * 128],
                        rhs=QT[
                            b * 64 : (b + 1) * 64,
                            h * 1024 + j * 512 : h * 1024 + (j + 1) * 512,
                        ],
                        start=True,
                        stop=True,
                    )
            # exp over the slot pair. Adjacent slots: ONE [128, 2048]
            # instruction (cheapest). Non-adjacent (the mod-3 wrap pair):
            # TWO per-slot instructions, so the read APs' bounding boxes
            # stay exact and don't falsely serialize against the third
            # slot's matmul writes (interval-based dependency tracking).
            eb = t * S
            if t not in DVE_TILES:
                if mx - mn == 1:
                    nc.scalar.activation(
                        E[:, eb : eb + S].rearrange("p (s c) -> p s c", c=1024),
                        PSv[:, mn : mx + 1, :],
                        mybir.ActivationFunctionType.Exp,
                        scale=SCALE,
                        accum_out=zs[:, t : t + 1],
                    )
                else:
                    for h in range(2):
                        slot = sa if h == 0 else sb
                        acc = (
                            zs[:, t : t + 1]
                            if h == 0
                            else zc2[:, t % 2 : t % 2 + 1]
                        )
                        nc.scalar.activation(
                            E[:, eb + eoff[(t, h)] : eb + eoff[(t, h)] + 1024],
                            PS[:, slot * 1024 : (slot + 1) * 1024],
                            mybir.ActivationFunctionType.Exp,
                            scale=SCALE,
                            accum_out=acc,
                        )
                    nc.vector.tensor_tensor(
                        zs[:, t : t + 1],
                        zs[:, t : t + 1],
                        zc2[:, t % 2 : t % 2 + 1],
                        mybir.AluOpType.add,
                    )
            else:
                if mx - mn == 1:
                    nc.vector.tensor_scalar(
                        Ei16[:, eb : eb + S].rearrange("p (s c) -> p s c", c=1024),
                        PSv[:, mn : mx + 1, :],
                        SCALE * A16,
                        BIAS16,
                        mybir.AluOpType.mult,
                        op1=mybir.AluOpType.add,
                    )
                else:
                    for h in range(2):
                        slot = sa if h == 0 else sb
                        nc.vector.tensor_scalar(
                            Ei16[
                                :, eb + eoff[(t, h)] : eb + eoff[(t, h)] + 1024
                            ],
                            PS[:, slot * 1024 : (slot + 1) * 1024],
                            SCALE * A16,
                            BIAS16,
                            mybir.AluOpType.mult,
                            op1=mybir.AluOpType.add,
                        )
                g = t % 2
                nc.gpsimd.tensor_tensor(
                    Tg[:, g * 1024 : (g + 1) * 1024],
                    E[:, eb : eb + 1024],
                    E[:, eb + 1024 : eb + 2048],
                    mybir.AluOpType.add,
                )
                nc.gpsimd.tensor_tensor(
                    Tg2[:, g * 512 : (g + 1) * 512],
                    Tg[:, g * 1024 : g * 1024 + 512],
                    Tg[:, g * 1024 + 512 : (g + 1) * 1024],
                    mybir.AluOpType.add,
                )
                nc.vector.tensor_reduce(
                    zs[:, t : t + 1],
                    Tg2[:, g * 512 : (g + 1) * 512],
                    mybir.AxisListType.X,
                    mybir.AluOpType.add,
                )
        # after odd chunks: finish the 4-tile group (2 chunks)
        if i % 2 == 1:
            g4 = (i - 1) * BPC
            nc.vector.reciprocal(rz[:, g4 : g4 + 4], zs[:, g4 : g4 + 4])
            for t in range(g4, g4 + 4):
                # V scaling: DVE tiles' Vs on ACT, ACT tiles' Vs on DVE
                if t in DVE_TILES:
                    nc.scalar.mul(
                        Vs[:, t * D : (t + 1) * D],
                        V[:, t * D : (t + 1) * D],
                        rz[:, t : t + 1],
                    )
                else:
                    nc.vector.tensor_scalar_mul(
                        Vs[:, t * D : (t + 1) * D],
                        V[:, t * D : (t + 1) * D],
                        rz[:, t : t + 1],
                    )
            for t in range(g4, g4 + 4):
                emit_av(pot0, zmm0, t, 0, stop_last=(t == NT - 1))

    # ---------------- tail: AV half 1, drains, transposes, stores ----------
    # pot1 reuses PS slot 0 (cols 0:1024); transpose scratch uses slot 2.
    pot1 = PS[:, 0:1024]
    zmm1 = []
    for j in range(2):
        zmm1.append(
            nc.tensor.matmul(
                pot1[:, j * 512 : (j + 1) * 512],
                lhsT=zw[:],
                rhs=QT[:, 0:512],
                start=True,
                stop=False,
                skip_group_check=True,
            )
        )
    # j=0 AVs for all tiles, then j=1 (so the j=0 drain can start early)
    for j in range(2):
        for t in range(NT):
            b = t % BPC
            base = t * S + eoff[(t, 1)]
            mm = nc.tensor.matmul(
                pot1[b * 64 : (b + 1) * 64, j * 512 : (j + 1) * 512],
                lhsT=Vs[:, t * D : (t + 1) * D],
                rhs=E[:, base + j * 512 : base + (j + 1) * 512],
                start=False,
                stop=(t == NT - 1),
                skip_group_check=True,
            )
            add_dep_helper(
                mm.ins, zmm1[j].ins, sync=False, reason="AV after zero matmul"
            )

    # drains: pot0 (free after its last AV), then pot1 halves
    for c in range(2):
        nc.scalar.copy(OT[:, c * 512 : (c + 1) * 512], pot0[:, c * 512 : (c + 1) * 512])
    for c in range(2):
        nc.scalar.copy(
            OT[:, 1024 + c * 512 : 1024 + (c + 1) * 512],
            pot1[:, c * 512 : (c + 1) * 512],
        )

    o_view = O_all[:].rearrange("p (m b d) -> p m b d", m=NCH, b=BPC, d=D)
    for grp in range(4):
        for m in range(4 * grp, 4 * grp + 4):
            ptc = PS[:, 2048 + (m % 4) * 128 : 2048 + (m % 4) * 128 + 128]
            nc.tensor.transpose(ptc, OT[:, m * 128 : (m + 1) * 128], ident[:])
            if m % 2 == 0:
                nc.vector.tensor_copy(O_all[:, m * 128 : (m + 1) * 128], ptc)
            else:
                nc.scalar.copy(O_all[:, m * 128 : (m + 1) * 128], ptc)
        for b in range(BPC):
            nc.sync.dma_start(
                o[b, 4 * grp * 128 : (4 * grp + 4) * 128, :].rearrange(
                    "(m p) d -> p m d", p=128
                ),
                o_view[:, 4 * grp : 4 * grp + 4, b, :],
            )


_CACHE: dict = {}


def build_program():
    if "nc" in _CACHE:
        return _CACHE["nc"]
    nc = bacc.Bacc("TRN2", target_bir_lowering=False, debug=False)
    q = nc.dram_tensor("q", [BPC, S, D], FP32, kind="ExternalInput").ap()
    k = nc.dram_tensor("k", [BPC, S, D], FP32, kind="ExternalInput").ap()
    v = nc.dram_tensor("v", [BPC, S, D], FP32, kind="ExternalInput").ap()
    o = nc.dram_tensor("o", [BPC, S, D], FP32, kind="ExternalOutput").ap()
    qbf_dram = nc.dram_tensor("qbf_dram", [S, 128], BF16, kind="Internal").ap()
    kbf_dram = nc.dram_tensor("kbf_dram", [S, 128], BF16, kind="Internal").ap()
    with tile.TileContext(nc) as tc:
        with ExitStack() as ctx:
            emit_kernel(ctx, tc, q, k, v, o, qbf_dram, kbf_dram)
    nc.compile()
    _CACHE["nc"] = nc
    return nc


def make_in_maps(q, k, v):
    q = np.ascontiguousarray(q, dtype=np.float32)
    k = np.ascontiguousarray(k, dtype=np.float32)
    v = np.ascontiguousarray(v, dtype=np.float32)
    assert q.shape == (B_FULL, S, D), q.shape
    return [
        {
            "q": np.ascontiguousarray(q[c * BPC : (c + 1) * BPC]),
            "k": np.ascontiguousarray(k[c * BPC : (c + 1) * BPC]),
            "v": np.ascontiguousarray(v[c * BPC : (c + 1) * BPC]),
        }
        for c in range(N_CORES)
    ]


def kernel(q, k, v, _trace=False):
    nc = build_program()
    in_maps = make_in_maps(q, k, v)
    res = bass_utils.run_bass_kernel_spmd(
        nc, in_maps, core_ids=list(range(N_CORES)), trace=_trace
    )
    out = np.concatenate([r["o"] for r in res.results], axis=0)
    if _trace:
        return out, res
    return out


# revision 14
# speedup vs baseline: 1.7317x; 1.7220x over previous
"""TRN2 Bass/Tile kernel for nn_DotProductAttention (softmax over the QUERY axis).

reference:
    scores  = einsum('bqd,bkd->bqk', q, k) / sqrt(64)
    weights = softmax(scores, axis=1)          # over q, NOT k!
    out     = einsum('bqk,bkd->bqd', weights, v)

Works with the transposed score matrix T = K @ Q^T ([k, q]): the softmax
reduction axis (q) is the free axis, and the normalizer Z[k] lives on the
contraction axis of the AV matmul so it folds into V (Vs = V / Z).

Sharding: B=16 batches, data-parallel over 8 cores => 2 batches per core,
packed into the two 64-partition halves of [128, *] tiles.

Per-core structure:
  phase A: f32 half loads (q on sync queue, k on scalar queue, both
    batches per DMA), bf16 casts on the idle DVE, bounce to DRAM [s, 128]
    and xbar DMA transpose into QT/KT [128 (b d), 2048 s]. V via SWDGE.
  phase B: per (k-chunk, batch) tile, two [128, 1024] score subtiles
    through a 3-deep PSUM pool; exp is split across engines:
    - 18 ACT tiles: activation-table exp per subtile, accum_out giving
      the two half-sums (combined into Z by one batched DVE add/group);
    - 14 DVE tiles: Schraudolph fast-exp (affine in f32, write the int16
      bit pattern of bf16, ~2% rms) per subtile, Z rebuilt by a 2-level
      GPSIMD half-add tree + a [128,512] DVE reduce.
    Per 2-chunk group: batched reciprocal, Vs = V * (1/Z), and the
    q-half-0 AV accumulation runs in the 2 spare PSUM banks.
  tail: AV for q-half 1, PSUM drains, 16 PE transposes of O^T, stores.
"""

import math
from contextlib import ExitStack

import numpy as np

import concourse.bass as bass  # noqa: F401
import concourse.mybir as mybir
import concourse.tile as tile
from bass_rust import add_dep_helper
from concourse import bacc, bass_utils
from concourse.masks import make_identity

FP32 = mybir.dt.float32
BF16 = mybir.dt.bfloat16
I16 = mybir.dt.int16

N_CORES = 8
B_FULL = 16
BPC = B_FULL // N_CORES  # batches per core = 2
S = 2048
D = 64
NCH = S // 128  # 16 key chunks of 128
NT = NCH * BPC  # 32 (chunk, batch) tiles
SCALE = 1.0 / math.sqrt(D)

# Schraudolph fast-exp constants for a bf16 bit pattern:
#   bf16_bits(exp(x)) ~= trunc(x * 128/ln2 + (127*128 - 6 + 0.5))
A16 = 128.0 / math.log(2.0)
BIAS16 = 127.0 * 128.0 - 6.0 + 0.5

# DVE-assigned tiles (14 of 32, evenly spread); ACT handles the rest.
DVE_TILES = frozenset(
    {1, 3, 5, 8, 10, 12, 15, 17, 19, 21, 24, 26, 28, 31}
)
# tiles whose Vs-scaling runs on ACT instead of DVE (load balance)
VS_ON_ACT = frozenset({1, 8, 15, 21, 28})


def emit_kernel(ctx: ExitStack, tc, q, k, v, o, qbf_dram, kbf_dram):
    nc = tc.nc

    const_pool = ctx.enter_context(tc.tile_pool(name="const", bufs=1))
    big = ctx.enter_context(tc.tile_pool(name="big", bufs=1))
    # PSUM: 3 rotating [128,1024] score subtile buffers (6 banks) +
    # pot0, the q-half-0 AV accumulator (2 banks).
    ps = ctx.enter_context(tc.tile_pool(name="ps", bufs=3, space="PSUM"))
    pp = ctx.enter_context(tc.tile_pool(name="pp", bufs=1, space="PSUM"))

    ident = const_pool.tile([128, 128], FP32, name="ident")
    make_identity(nc, ident)
    identb = const_pool.tile([128, 128], BF16, name="identb")
    make_identity(nc, identb)
    zw = const_pool.tile([128, 128], BF16, name="zw")
    nc.vector.memset(zw[:], 0.0)

    # (b,d)-packed transposed operands: partitions 0:64 = batch0 d, 64:128 = b1.
    QT = big.tile([128, S], BF16, name="QT")
    KT = big.tile([128, S], BF16, name="KT")
    qstage = big.tile([128, S], FP32, name="qstage")
    kstage = big.tile([128, S], FP32, name="kstage")
    qbf = big.tile([128, S], BF16, name="qbf")
    kbf = big.tile([128, S], BF16, name="kbf")
    # V chunks [128 k, (t d)] f32 and Vs = V / Z (bf16); t = i*BPC + b
    V = big.tile([128, NT * D], FP32, name="V")
    Vs = big.tile([128, NT * D], BF16, name="Vs")
    # E[t*S :+ S] = exp(scores*SCALE): [128 k, 2048 q] bf16, fully resident
    E = big.tile([128, NT * S], BF16, name="E")
    Ei16 = E[:].bitcast(I16)
    # Z bookkeeping: zs = first-half sums (ACT) / full sums (DVE tiles);
    # zcb = second-half sums (ACT tiles), zero elsewhere; zs += zcb per
    # group, then rz = 1/zs.
    zs = big.tile([128, NT], FP32, name="zs")
    zcb = big.tile([128, NT], FP32, name="zcb")
    rz = big.tile([128, NT], FP32, name="rz")
    nc.vector.memset(zcb[:], 0.0)
    # gpsimd half-add scratch (2-level tree, rotating pair)
    Tg = big.tile([128, 2 * 1024], FP32, name="Tg")
    Tg2 = big.tile([128, 2 * 512], FP32, name="Tg2")
    # O^T staging ((b,d) packed on partitions, q on free), f32
    OT = big.tile([128, S], FP32, name="OT")
    # O in natural layout: column chunk m holds [q-tile m, (b d)]
    O_all = big.tile([128, S], FP32, name="O_all")

    # ---------------- phase A: load + transpose Q/K, load V ----------------
    # Proven baseline structure: quarter-granularity stage loads on the sync
    # queue, DVE casts, fast PE transposes for q0..15 + k0..3 (what B1 needs
    # first), one whole-tensor xbar DMA (bf16 DRAM roundtrip) for KT 4..15,
    # V on the otherwise-idle SWDGE path.
    QRT = NCH // 4
    last_q_dma = None
    for src, stg, Q in (
        (q, qstage, 0),
        (q, qstage, 1),
        (k, kstage, 0),
        (q, qstage, 2),
        (q, qstage, 3),
        (k, kstage, 1),
        (k, kstage, 2),
        (k, kstage, 3),
    ):
        ssl = slice(Q * QRT * 128, (Q + 1) * QRT * 128)
        for b in range(BPC):
            dma = nc.sync.dma_start(
                stg[:, ssl].rearrange("p (m b d) -> p m b d", m=QRT, b=BPC, d=D)[
                    :, :, b, :
                ],
                src[b, ssl, :].rearrange("(m p) d -> p m d", p=128),
            )
            if src is q:
                last_q_dma = dma
    for b in range(BPC):
        vdma = nc.gpsimd.dma_start(
            V[:].rearrange("p (i b d) -> p i b d", i=NCH, b=BPC)[:, :, b, :],
            v[b].rearrange("(i p) d -> p i d", p=128),
        )
        add_dep_helper(
            vdma.ins, last_q_dma.ins, sync=True, reason="delay V behind q loads"
        )
    for Q in range(4):
        csl = slice(Q * QRT * 128, (Q + 1) * QRT * 128)
        nc.vector.tensor_copy(qbf[:, csl], qstage[:, csl])
        nc.vector.tensor_copy(kbf[:, csl], kstage[:, csl])
    for idx, (tt, m) in enumerate(
        [("q", mm) for mm in range(8)]
        + [("k", mm) for mm in range(4)]
        + [("q", mm) for mm in range(8, NCH)]
    ):
        bft, dst = (qbf, QT) if tt == "q" else (kbf, KT)
        pt = ps.tile([128, 128], BF16, tag="ps", name=f"pt_{tt}{m}")
        nc.tensor.transpose(pt[:], bft[:, m * 128 : (m + 1) * 128], identb[:])
        if idx % 2 == 0:
            nc.scalar.copy(dst[:, m * 128 : (m + 1) * 128], pt[:])
        else:
            nc.vector.tensor_copy(dst[:, m * 128 : (m + 1) * 128], pt[:])
    # KT chunks 4..15 via DRAM-roundtrip whole-tensor xbar transpose
    nc.sync.dma_start(
        kbf_dram[512:S, :].rearrange("(m p) c -> p m c", p=128),
        kbf[:, 512:S].rearrange("p (m c) -> p m c", m=NCH - 4),
    )
    nc.sync.dma_start_transpose(out=KT[:, 512:S], in_=kbf_dram[512:S, :])

    # pot0: open every (b, j) region with a zeroing matmul so the
    # partition-sliced AV matmuls can accumulate with start=False.
    pot0 = pp.tile([128, 1024], FP32, tag="pot", name="pot0")
    zmm0 = []
    for j in range(2):
        zmm0.append(
            nc.tensor.matmul(
                pot0[:, j * 512 : (j + 1) * 512],
                lhsT=zw[:],
                rhs=QT[:, 0:512],
                start=True,
                stop=False,
                skip_group_check=True,
            )
        )

    # ---------------- phase B1: scores -> exp (+Z), AV half 0 --------------
    def emit_av(pot, zmm, t, h, stop_last):
        b = t % BPC
        base = t * S + h * 1024
        for j in range(2):
            mm = nc.tensor.matmul(
                pot[b * 64 : (b + 1) * 64, j * 512 : (j + 1) * 512],
                lhsT=Vs[:, t * D : (t + 1) * D],
                rhs=E[:, base + j * 512 : base + (j + 1) * 512],
                start=False,
                stop=stop_last,
                skip_group_check=True,
            )
            if zmm is not None:
                add_dep_helper(
                    mm.ins,
                    zmm[j].ins,
                    sync=False,
                    reason="AV after bank-opening zero matmul",
                )

    dve_ord = [0]
    for i in range(NCH):
        for b in range(BPC):
            t = i * BPC + b
            dve = t in DVE_TILES
            for h in range(2):
                sct = ps.tile([128, 1024], FP32, tag="ps", name=f"sc{t}_{h}")
                for j in range(2):
                    nc.tensor.matmul(
                        sct[:, j * 512 : (j + 1) * 512],
                        lhsT=KT[b * 64 : (b + 1) * 64, i * 128 : (i + 1) * 128],
                        rhs=QT[
                            b * 64 : (b + 1) * 64,
                            h * 1024 + j * 512 : h * 1024 + (j + 1) * 512,
                        ],
                        start=True,
                        stop=True,
                    )
                eb = t * S + h * 1024
                if not dve:
                    acc = zs[:, t : t + 1] if h == 0 else zcb[:, t : t + 1]
                    nc.scalar.activation(
                        E[:, eb : eb + 1024],
                        sct[:],
                        mybir.ActivationFunctionType.Exp,
                        scale=SCALE,
                        accum_out=acc,
                    )
                else:
                    nc.vector.tensor_scalar(
                        Ei16[:, eb : eb + 1024],
                        sct[:],
                        SCALE * A16,
                        BIAS16,
                        mybir.AluOpType.mult,
                        op1=mybir.AluOpType.add,
                    )
            if dve:
                g = dve_ord[0] % 2
                dve_ord[0] += 1
                eb = t * S
                nc.gpsimd.tensor_tensor(
                    Tg[:, g * 1024 : (g + 1) * 1024],
                    E[:, eb : eb + 1024],
                    E[:, eb + 1024 : eb + 2048],
                    mybir.AluOpType.add,
                )
                nc.gpsimd.tensor_tensor(
                    Tg2[:, g * 512 : (g + 1) * 512],
                    Tg[:, g * 1024 : g * 1024 + 512],
                    Tg[:, g * 1024 + 512 : (g + 1) * 1024],
                    mybir.AluOpType.add,
                )
                nc.vector.tensor_reduce(
                    zs[:, t : t + 1],
                    Tg2[:, g * 512 : (g + 1) * 512],
                    mybir.AxisListType.X,
                    mybir.AluOpType.add,
                )
        # after odd chunks: finish the 4-tile group (2 chunks)
        if i % 2 == 1:
            g4 = (i - 1) * BPC
            nc.vector.tensor_tensor(
                zs[:, g4 : g4 + 4],
                zs[:, g4 : g4 + 4],
                zcb[:, g4 : g4 + 4],
                mybir.AluOpType.add,
            )
            nc.vector.reciprocal(rz[:, g4 : g4 + 4], zs[:, g4 : g4 + 4])
            for t in range(g4, g4 + 4):
                if t in VS_ON_ACT:
                    nc.scalar.mul(
                        Vs[:, t * D : (t + 1) * D],
                        V[:, t * D : (t + 1) * D],
                        rz[:, t : t + 1],
                    )
                else:
                    nc.vector.tensor_scalar_mul(
                        Vs[:, t * D : (t + 1) * D],
                        V[:, t * D : (t + 1) * D],
                        rz[:, t : t + 1],
                    )
            for t in range(g4, g4 + 4):
                emit_av(pot0, zmm0, t, 0, stop_last=(t == NT - 1))

    # ---------------- tail: AV half 1, drains, transposes, stores ----------
    pot1 = ps.tile([128, 1024], FP32, tag="ps", name="pot1")
    zmm1 = []
    for j in range(2):
        zmm1.append(
            nc.tensor.matmul(
                pot1[:, j * 512 : (j + 1) * 512],
                lhsT=zw[:],
                rhs=QT[:, 0:512],
                start=True,
                stop=False,
                skip_group_check=True,
            )
        )
    # j=0 AVs for all tiles, then j=1 (so the j=0 drain can start early)
    for j in range(2):
        for t in range(NT):
            b = t % BPC
            base = t * S + 1024
            mm = nc.tensor.matmul(
                pot1[b * 64 : (b + 1) * 64, j * 512 : (j + 1) * 512],
                lhsT=Vs[:, t * D : (t + 1) * D],
                rhs=E[:, base + j * 512 : base + (j + 1) * 512],
                start=False,
                stop=(t == NT - 1),
                skip_group_check=True,
            )
            add_dep_helper(
                mm.ins, zmm1[j].ins, sync=False, reason="AV after zero matmul"
            )

    # drains: pot0 (free after its last AV), then pot1 halves
    for c in range(2):
        nc.scalar.copy(OT[:, c * 512 : (c + 1) * 512], pot0[:, c * 512 : (c + 1) * 512])
    for c in range(2):
        nc.scalar.copy(
            OT[:, 1024 + c * 512 : 1024 + (c + 1) * 512],
            pot1[:, c * 512 : (c + 1) * 512],
        )

    o_view = O_all[:].rearrange("p (m b d) -> p m b d", m=NCH, b=BPC, d=D)
    for grp in range(4):
        for m in range(4 * grp, 4 * grp + 4):
            ptc = ps.tile([128, 128], FP32, tag="ps", name=f"ptc_{m}")
            nc.tensor.transpose(ptc[:], OT[:, m * 128 : (m + 1) * 128], ident[:])
            if m % 2 == 0:
                nc.vector.tensor_copy(O_all[:, m * 128 : (m + 1) * 128], ptc[:])
            else:
                nc.scalar.copy(O_all[:, m * 128 : (m + 1) * 128], ptc[:])
        for b in range(BPC):
            nc.sync.dma_start(
                o[b, 4 * grp * 128 : (4 * grp + 4) * 128, :].rearrange(
                    "(m p) d -> p m d", p=128
                ),
                o_view[:, 4 * grp : 4 * grp + 4, b, :],
            )


_CACHE: dict = {}


def build_program():
    if "nc" in _CACHE:
        return _CACHE["nc"]
    nc = bacc.Bacc("TRN2", target_bir_lowering=False, debug=False)
    q = nc.dram_tensor("q", [BPC, S, D], FP32, kind="ExternalInput").ap()
    k = nc.dram_tensor("k", [BPC, S, D], FP32, kind="ExternalInput").ap()
    v = nc.dram_tensor("v", [BPC, S, D], FP32, kind="ExternalInput").ap()
    o = nc.dram_tensor("o", [BPC, S, D], FP32, kind="ExternalOutput").ap()
    qbf_dram = nc.dram_tensor("qbf_dram", [S, 128], BF16, kind="Internal").ap()
    kbf_dram = nc.dram_tensor("kbf_dram", [S, 128], BF16, kind="Internal").ap()
    with tile.TileContext(nc) as tc:
        with ExitStack() as ctx:
            emit_kernel(ctx, tc, q, k, v, o, qbf_dram, kbf_dram)
    nc.compile()
    _CACHE["nc"] = nc
    return nc


def make_in_maps(q, k, v):
    q = np.ascontiguousarray(q, dtype=np.float32)
    k = np.ascontiguousarray(k, dtype=np.float32)
    v = np.ascontiguousarray(v, dtype=np.float32)
    assert q.shape == (B_FULL, S, D), q.shape
    return [
        {
            "q": np.ascontiguousarray(q[c * BPC : (c + 1) * BPC]),
            "k": np.ascontiguousarray(k[c * BPC : (c + 1) * BPC]),
            "v": np.ascontiguousarray(v[c * BPC : (c + 1) * BPC]),
        }
        for c in range(N_CORES)
    ]


def kernel(q, k, v, _trace=False):
    nc = build_program()
    in_maps = make_in_maps(q, k, v)
    res = bass_utils.run_bass_kernel_spmd(
        nc, in_maps, core_ids=list(range(N_CORES)), trace=_trace
    )
    out = np.concatenate([r["o"] for r in res.results], axis=0)
    if _trace:
        return out, res
    return out
